# revision 1
# baseline (speedup 1.0000x reference)
"""Trainium2 Bass kernel for a 6-layer GRU network (B=256, T=512, I=28, H=128, O=10).

Strategy: data-parallel across 8 NeuronCores (batch 256 -> 32 per core).
Per core, everything lives in "transposed" layout: partitions = hidden/gate
dim, free dim = time*batch. Per layer:
  - input projections gx = W_ih^T.T @ h_prev_seq computed as chunked GEMMs
    directly into PSUM (one bank per gate per chunk),
  - the sequential recurrence accumulates gh_r/gh_z onto gx_r/gx_z in PSUM
    via start=False matmuls; the n-gate recurrent part goes to a separate
    PSUM tile so r can gate it,
  - gates: sigmoid/tanh on ScalarE (per-layer biases ride the free ACT bias
    port), (gh_n + b_hh_n) * r fused into one scalar_tensor_tensor on DVE,
  - h_new = n + z*(h - n) on DVE, written into per-chunk h-seq SBUF tiles
    that the next layer's GEMM consumes.
Final: logits = h_last^T.T @ fc_w^T + fc_b (fc_b added via a K=1 accumulate
matmul), then log_softmax along the free dim.
"""

import numpy as np

H = 128
I_DIM = 28
L = 6
O = 10
B = 256
T = 512
NCORES = 8
PB = B // NCORES  # 32 batch rows per core
C = 16            # timesteps per chunk (C*PB = 512 = one PSUM bank of fp32)

_CACHE = {}


def _build(t_steps, dt_mm_name="float32"):
    from contextlib import ExitStack

    import concourse.bass as bass  # noqa: F401
    import concourse.tile as tile
    from concourse import bacc, mybir

    f32 = mybir.dt.float32
    dt_mm = getattr(mybir.dt, dt_mm_name)
    AF = mybir.ActivationFunctionType
    ALU = mybir.AluOpType

    n_chunks = t_steps // C
    assert n_chunks * C == t_steps

    nc = bacc.Bacc("TRN2", target_bir_lowering=False, debug=False)

    xT = nc.dram_tensor("xT", [I_DIM, PB * t_steps], dt_mm, kind="ExternalInput")
    wih0 = nc.dram_tensor("wih0", [I_DIM, 3 * H], dt_mm, kind="ExternalInput")
    wih = nc.dram_tensor("wih", [H, (L - 1) * 3 * H], dt_mm, kind="ExternalInput")
    whh = nc.dram_tensor("whh", [H, L * 3 * H], dt_mm, kind="ExternalInput")
    bias_r = nc.dram_tensor("bias_r", [H, L], f32, kind="ExternalInput")
    bias_z = nc.dram_tensor("bias_z", [H, L], f32, kind="ExternalInput")
    bihn = nc.dram_tensor("bihn", [H, L], f32, kind="ExternalInput")
    bhhn = nc.dram_tensor("bhhn", [H, L], f32, kind="ExternalInput")
    fcw = nc.dram_tensor("fcw", [H, O], dt_mm, kind="ExternalInput")
    fcb = nc.dram_tensor("fcb", [1, O], dt_mm, kind="ExternalInput")
    y = nc.dram_tensor("y", [PB, O], f32, kind="ExternalOutput")

    with tile.TileContext(nc) as tc, ExitStack() as ctx:
        consts = ctx.enter_context(tc.tile_pool(name="consts", bufs=1))
        hseq_pool = ctx.enter_context(tc.tile_pool(name="hseq", bufs=2 * n_chunks))
        gxr_pool = ctx.enter_context(tc.tile_pool(name="gxr", bufs=2, space="PSUM"))
        gxz_pool = ctx.enter_context(tc.tile_pool(name="gxz", bufs=2, space="PSUM"))
        gxn_pool = ctx.enter_context(tc.tile_pool(name="gxn", bufs=2, space="PSUM"))
        ps_small = ctx.enter_context(tc.tile_pool(name="ps_small", bufs=2, space="PSUM"))
        scratch = ctx.enter_context(tc.tile_pool(name="scratch", bufs=3))

        # --- load constants/weights ---
        xT_sb = consts.tile([I_DIM, PB * t_steps], dt_mm, tag="xT_sb")
        nc.gpsimd.dma_start(xT_sb[:], xT.ap())
        wih0_sb = consts.tile([I_DIM, 3 * H], dt_mm, tag="wih0_sb")
        nc.gpsimd.dma_start(wih0_sb[:], wih0.ap())
        wih_sb = consts.tile([H, (L - 1) * 3 * H], dt_mm, tag="wih_sb")
        nc.gpsimd.dma_start(wih_sb[:], wih.ap())
        whh_sb = consts.tile([H, L * 3 * H], dt_mm, tag="whh_sb")
        nc.gpsimd.dma_start(whh_sb[:], whh.ap())
        bias_r_sb = consts.tile([H, L], f32, tag="bias_r_sb")
        nc.gpsimd.dma_start(bias_r_sb[:], bias_r.ap())
        bias_z_sb = consts.tile([H, L], f32, tag="bias_z_sb")
        nc.gpsimd.dma_start(bias_z_sb[:], bias_z.ap())
        bihn_sb = consts.tile([H, L], f32, tag="bihn_sb")
        nc.gpsimd.dma_start(bihn_sb[:], bihn.ap())
        bhhn_sb = consts.tile([H, L], f32, tag="bhhn_sb")
        nc.gpsimd.dma_start(bhhn_sb[:], bhhn.ap())
        fcw_sb = consts.tile([H, O], dt_mm, tag="fcw_sb")
        nc.gpsimd.dma_start(fcw_sb[:], fcw.ap())
        fcb_sb = consts.tile([1, O], dt_mm, tag="fcb_sb")
        nc.gpsimd.dma_start(fcb_sb[:], fcb.ap())

        zeros_sb = consts.tile([H, PB], dt_mm, tag="zeros_sb")
        nc.vector.memset(zeros_sb[:], 0.0)
        ones_sb = consts.tile([1, PB], dt_mm, tag="ones_sb")
        nc.vector.memset(ones_sb[:], 1.0)

        def whh_g(layer, g):
            return whh_sb[:, (layer * 3 + g) * H:(layer * 3 + g + 1) * H]

        def wih_g(layer, g):
            assert layer >= 1
            base = ((layer - 1) * 3 + g) * H
            return wih_sb[:, base:base + H]

        prev_chunks = None  # list of SBUF tiles [H, C*PB] for layer l-1 output
        h_last = None
        for layer in range(L):
            cur_chunks = []
            h_prev = zeros_sb[:, :]
            for k in range(n_chunks):
                # --- input-projection GEMM for this chunk (into PSUM) ---
                gxr_t = gxr_pool.tile([H, C * PB], f32)
                gxz_t = gxz_pool.tile([H, C * PB], f32)
                gxn_t = gxn_pool.tile([H, C * PB], f32)
                if layer == 0:
                    mv = xT_sb[:, k * C * PB:(k + 1) * C * PB]
                    lhs = [wih0_sb[:, g * H:(g + 1) * H] for g in range(3)]
                else:
                    mv = prev_chunks[k][:, :]
                    lhs = [wih_g(layer, g) for g in range(3)]
                nc.tensor.matmul(gxr_t[:], lhs[0], mv, start=True, stop=False)
                nc.tensor.matmul(gxz_t[:], lhs[1], mv, start=True, stop=False)
                nc.tensor.matmul(gxn_t[:], lhs[2], mv, start=True, stop=True)

                hcur_t = hseq_pool.tile([H, C * PB], dt_mm)
                cur_chunks.append(hcur_t)

                for s in range(C):
                    sl = slice(s * PB, (s + 1) * PB)
                    # recurrent matmuls
                    nc.tensor.matmul(gxr_t[:, sl], whh_g(layer, 0), h_prev,
                                     start=False, stop=(s == C - 1),
                                     skip_group_check=True)
                    nc.tensor.matmul(gxz_t[:, sl], whh_g(layer, 1), h_prev,
                                     start=False, stop=(s == C - 1),
                                     skip_group_check=True)
                    ghn_t = ps_small.tile([H, PB], f32, tag="ghn")
                    nc.tensor.matmul(ghn_t[:], whh_g(layer, 2), h_prev,
                                     start=True, stop=True)
                    # gates
                    r_t = scratch.tile([H, PB], f32, tag="r")
                    nc.scalar.activation(r_t[:], gxr_t[:, sl], AF.Sigmoid,
                                         bias=bias_r_sb[:, layer:layer + 1])
                    z_t = scratch.tile([H, PB], f32, tag="z")
                    nc.scalar.activation(z_t[:], gxz_t[:, sl], AF.Sigmoid,
                                         bias=bias_z_sb[:, layer:layer + 1])
                    hn2_t = scratch.tile([H, PB], f32, tag="hn2")
                    nc.vector.scalar_tensor_tensor(
                        hn2_t[:], ghn_t[:], bhhn_sb[:, layer:layer + 1], r_t[:],
                        op0=ALU.add, op1=ALU.mult)
                    nin_t = scratch.tile([H, PB], f32, tag="nin")
                    nc.vector.tensor_tensor(nin_t[:], gxn_t[:, sl], hn2_t[:],
                                            op=ALU.add)
                    n_t = scratch.tile([H, PB], f32, tag="n")
                    nc.scalar.activation(n_t[:], nin_t[:], AF.Tanh,
                                         bias=bihn_sb[:, layer:layer + 1])
                    d_t = scratch.tile([H, PB], f32, tag="d")
                    nc.vector.tensor_tensor(d_t[:], h_prev, n_t[:],
                                            op=ALU.subtract)
                    e_t = scratch.tile([H, PB], f32, tag="e")
                    nc.vector.tensor_tensor(e_t[:], z_t[:], d_t[:], op=ALU.mult)
                    h_new = hcur_t[:, sl]
                    nc.vector.tensor_tensor(h_new, n_t[:], e_t[:], op=ALU.add)
                    h_prev = h_new
            prev_chunks = cur_chunks
            h_last = h_prev

        # --- FC + log_softmax on the last timestep of the last layer ---
        logits_ps = ps_small.tile([PB, O], f32, tag="ghn")
        nc.tensor.matmul(logits_ps[:], h_last, fcw_sb[:], start=True, stop=False)
        nc.tensor.matmul(logits_ps[:], ones_sb[:], fcb_sb[:],
                         start=False, stop=True, skip_group_check=True)
        mx_t = scratch.tile([PB, 1], f32, tag="mx")
        nc.vector.reduce_max(mx_t[:], logits_ps[:], axis=mybir.AxisListType.X)
        xm_t = scratch.tile([PB, O], f32, tag="xm")
        nc.vector.tensor_scalar(xm_t[:], logits_ps[:], mx_t[:], None,
                                op0=ALU.subtract)
        ex_t = scratch.tile([PB, O], f32, tag="ex")
        sum_t = scratch.tile([PB, 1], f32, tag="sum")
        nc.scalar.activation(ex_t[:], xm_t[:], AF.Exp, accum_out=sum_t[:])
        ls_t = scratch.tile([PB, 1], f32, tag="ls")
        nc.scalar.activation(ls_t[:], sum_t[:], AF.Ln)
        out_t = scratch.tile([PB, O], f32, tag="out")
        nc.vector.tensor_scalar(out_t[:], xm_t[:], ls_t[:], None,
                                op0=ALU.subtract)
        nc.gpsimd.dma_start(y.ap(), out_t[:])

    nc.compile()
    return nc


def _prep_inputs(x, W_ih0, W_ih_rest, W_hh, b_ih, b_hh, fc_w, fc_b, t_steps,
                 np_mm=np.float32):
    """Host-side reshape/transpose into the layouts the kernel expects."""
    f = np.float32
    b_ih = np.asarray(b_ih, f)
    b_hh = np.asarray(b_hh, f)
    shared = {
        "wih0": np.ascontiguousarray(np.asarray(W_ih0, f).T.astype(np_mm)),
        "wih": np.ascontiguousarray(
            np.concatenate([np.asarray(W_ih_rest[l], f).T for l in range(L - 1)],
                           axis=1).astype(np_mm)),
        "whh": np.ascontiguousarray(
            np.concatenate([np.asarray(W_hh[l], f).T for l in range(L)],
                           axis=1).astype(np_mm)),
        "bias_r": np.ascontiguousarray((b_ih[:, 0:H] + b_hh[:, 0:H]).T),
        "bias_z": np.ascontiguousarray((b_ih[:, H:2 * H] + b_hh[:, H:2 * H]).T),
        "bihn": np.ascontiguousarray(b_ih[:, 2 * H:3 * H].T),
        "bhhn": np.ascontiguousarray(b_hh[:, 2 * H:3 * H].T),
        "fcw": np.ascontiguousarray(np.asarray(fc_w, f).T.astype(np_mm)),
        "fcb": np.ascontiguousarray(np.asarray(fc_b, f).reshape(1, O).astype(np_mm)),
    }
    x = np.asarray(x, f)[:, :t_steps, :]
    in_maps = []
    for c in range(NCORES):
        xc = x[c * PB:(c + 1) * PB]                      # [PB, t, I]
        xT_c = np.ascontiguousarray(xc.transpose(2, 1, 0).reshape(I_DIM, t_steps * PB).astype(np_mm))
        in_maps.append({"xT": xT_c, **shared})
    return in_maps


def _run(nc, in_maps, trace=False):
    from concourse.bass_utils import run_bass_kernel_spmd
    return run_bass_kernel_spmd(nc, in_maps, core_ids=list(range(NCORES)),
                                trace=trace)


def kernel(x, W_ih0, W_ih_rest, W_hh, b_ih, b_hh, fc_w, fc_b):
    import ml_dtypes
    key = ("bf16", T)
    if key not in _CACHE:
        _CACHE[key] = _build(T, "bfloat16")
    nc = _CACHE[key]
    in_maps = _prep_inputs(x, W_ih0, W_ih_rest, W_hh, b_ih, b_hh, fc_w, fc_b, T,
                           np_mm=ml_dtypes.bfloat16)
    res = _run(nc, in_maps)
    return np.concatenate([res.results[c]["y"] for c in range(NCORES)], axis=0)



# revision 9
# speedup vs baseline: 3.4396x; 3.4396x over previous
"""Trainium2 Bass kernel for a 6-layer GRU network (B=256, T=512, I=28, H=128, O=10).

Strategy: data-parallel across 8 NeuronCores (batch 256 -> 32 per core),
with a 6-layer WAVEFRONT schedule inside each core: at wavefront step w,
layer l processes timestep t = w - 8*l.  The six layers are split into two
independent groups of three (layers 0-2 / 3-5) whose dependency chains
interleave on the engines, and all gate elementwise work is batched across
each group's three layers into [128, 96]-wide ops (vs [128, 32] per-layer).

Per group-step:
  - PSUM "rz" tile [128, 2steps x 3layers x 2gates x 32] accumulates
    bias (K=6 selector matmul, start=True) + input projection (chunked,
    strided dest, start=False) + recurrent W_hh matmuls (start=False) so
    ONE sigmoid op reads a contiguous [128,192] tile and emits bf16 SBUF.
  - n-gate: gxn PSUM tile (bias + input proj), ghn PSUM tile (bias +
    recurrent mm); hn2 = ghn * r and nin = hn2 + gxn on GpSimd; tanh on
    ScalarE; h-update (d = h-n, e = z*d, h = n+e) on DVE in bf16 SBUF
    (4x fast mode).
  - h state lives in per-layer SBUF rings [128, L, 16, 32] indexed by
    wavefront slot (w % 16), so the batched 3-layer h-update writes one
    strided AP.
Final FC + log_softmax identical to the data-parallel baseline.
"""

import numpy as np

H = 128
I_DIM = 28
L = 6
O = 10
B = 256
T = 512
NCORES = 8
PB = B // NCORES   # 32 batch rows per core
D_OFF = 8          # wavefront offset between consecutive layers
RING = 16          # h-state ring depth (slots of PB cols per layer)
GRPS = ([0, 1, 2], [3, 4, 5])

_CACHE = {}


def _build(t_steps, dt_mm_name="bfloat16"):
    from contextlib import ExitStack

    import concourse.bass as bass  # noqa: F401
    import concourse.tile as tile
    from concourse import bacc, mybir

    f32 = mybir.dt.float32
    bf16 = mybir.dt.bfloat16
    dt_mm = getattr(mybir.dt, dt_mm_name)
    AF = mybir.ActivationFunctionType
    ALU = mybir.AluOpType

    assert t_steps % 2 == 0
    w_end = t_steps + (L - 1) * D_OFF  # wavefront length

    nc = bacc.Bacc("TRN2", target_bir_lowering=False, debug=False)

    xT = nc.dram_tensor("xT", [I_DIM, PB * t_steps], dt_mm, kind="ExternalInput")
    wih0 = nc.dram_tensor("wih0", [I_DIM, 3 * H], dt_mm, kind="ExternalInput")
    wih = nc.dram_tensor("wih", [H, (L - 1) * 3 * H], dt_mm, kind="ExternalInput")
    whh = nc.dram_tensor("whh", [H, L * 3 * H], dt_mm, kind="ExternalInput")
    # rz bias rows per group: [6, H] (row k = layer grp[k//2], gate k%2 (r/z))
    brz_a = nc.dram_tensor("brz_a", [6, H], dt_mm, kind="ExternalInput")
    brz_b = nc.dram_tensor("brz_b", [6, H], dt_mm, kind="ExternalInput")
    erz = nc.dram_tensor("erz", [6, 2 * 3 * 2 * PB], dt_mm, kind="ExternalInput")
    bihn_a = nc.dram_tensor("bihn_a", [3, H], dt_mm, kind="ExternalInput")
    bihn_b = nc.dram_tensor("bihn_b", [3, H], dt_mm, kind="ExternalInput")
    e3 = nc.dram_tensor("e3", [3, 2 * 3 * PB], dt_mm, kind="ExternalInput")
    bhhn_a = nc.dram_tensor("bhhn_a", [3, H], dt_mm, kind="ExternalInput")
    bhhn_b = nc.dram_tensor("bhhn_b", [3, H], dt_mm, kind="ExternalInput")
    e3b = nc.dram_tensor("e3b", [3, 3 * PB], dt_mm, kind="ExternalInput")
    fcw = nc.dram_tensor("fcw", [H, O], dt_mm, kind="ExternalInput")
    fcb = nc.dram_tensor("fcb", [1, O], dt_mm, kind="ExternalInput")
    y = nc.dram_tensor("y", [PB, O], f32, kind="ExternalOutput")

    with tile.TileContext(nc) as tc, ExitStack() as ctx:
        consts = ctx.enter_context(tc.tile_pool(name="consts", bufs=1))
        # One persistent PSUM pool per group: rz ring (2 banks) + gxn ring
        # (1 bank) + ghn ping-pong (0.5 bank) = 4 banks; x2 groups = 8 banks.
        # Ring slots are padded so no matmul dest window crosses a bank.
        ps_pool = [
            ctx.enter_context(tc.tile_pool(name=f"ps_pool{g}", bufs=1, space="PSUM"))
            for g in range(2)
        ]
        rz_t = []
        gxn_t = []
        ghn_t = []
        for g in range(2):
            rz = ps_pool[g].tile([H, 4, 8, PB], f32, tag=f"rz{g}", name=f"rz{g}")
            gxn = ps_pool[g].tile([H, 4, 4, PB], f32, tag=f"gxn{g}", name=f"gxn{g}")
            ghn = ps_pool[g].tile([H, 2, 4, PB], f32, tag=f"ghn{g}", name=f"ghn{g}")
            rz_t.append(rz)
            gxn_t.append(gxn)
            ghn_t.append(ghn)
        rzsb_pool = ctx.enter_context(tc.tile_pool(name="rzsb", bufs=3))
        ew_pool = ctx.enter_context(tc.tile_pool(name="ew", bufs=3))
        scratch = ctx.enter_context(tc.tile_pool(name="scratch", bufs=3))

        # ---- load constants ----
        xT_sb = consts.tile([I_DIM, PB * t_steps], dt_mm, tag="xT_sb")
        nc.gpsimd.dma_start(xT_sb[:], xT.ap())
        wih0_sb = consts.tile([I_DIM, 3 * H], dt_mm, tag="wih0_sb")
        nc.gpsimd.dma_start(wih0_sb[:], wih0.ap())
        wih_sb = consts.tile([H, (L - 1) * 3 * H], dt_mm, tag="wih_sb")
        nc.gpsimd.dma_start(wih_sb[:], wih.ap())
        whh_sb = consts.tile([H, L * 3 * H], dt_mm, tag="whh_sb")
        nc.gpsimd.dma_start(whh_sb[:], whh.ap())
        brz_sb = [consts.tile([6, H], dt_mm, tag=f"brz{g}_sb", name=f"brz{g}_sb") for g in range(2)]
        nc.gpsimd.dma_start(brz_sb[0][:], brz_a.ap())
        nc.gpsimd.dma_start(brz_sb[1][:], brz_b.ap())
        erz_sb = consts.tile([6, 2 * 3 * 2 * PB], dt_mm, tag="erz_sb")
        nc.gpsimd.dma_start(erz_sb[:], erz.ap())
        bihn_sb = [consts.tile([3, H], dt_mm, tag=f"bihn{g}_sb", name=f"bihn{g}_sb") for g in range(2)]
        nc.gpsimd.dma_start(bihn_sb[0][:], bihn_a.ap())
        nc.gpsimd.dma_start(bihn_sb[1][:], bihn_b.ap())
        e3_sb = consts.tile([3, 2 * 3 * PB], dt_mm, tag="e3_sb")
        nc.gpsimd.dma_start(e3_sb[:], e3.ap())
        bhhn_sb = [consts.tile([3, H], dt_mm, tag=f"bhhn{g}_sb", name=f"bhhn{g}_sb") for g in range(2)]
        nc.gpsimd.dma_start(bhhn_sb[0][:], bhhn_a.ap())
        nc.gpsimd.dma_start(bhhn_sb[1][:], bhhn_b.ap())
        e3b_sb = consts.tile([3, 3 * PB], dt_mm, tag="e3b_sb")
        nc.gpsimd.dma_start(e3b_sb[:], e3b.ap())
        fcw_sb = consts.tile([H, O], dt_mm, tag="fcw_sb")
        nc.gpsimd.dma_start(fcw_sb[:], fcw.ap())
        fcb_sb = consts.tile([1, O], dt_mm, tag="fcb_sb")
        nc.gpsimd.dma_start(fcb_sb[:], fcb.ap())

        zeros_sb = consts.tile([H, PB], dt_mm, tag="zeros_sb")
        nc.vector.memset(zeros_sb[:], 0.0)
        ones_sb = consts.tile([1, PB], dt_mm, tag="ones_sb")
        nc.vector.memset(ones_sb[:], 1.0)

        # h-state rings: [128, layer, slot, batch]; slot = w % RING
        h_ring = consts.tile([H, L, RING, PB], dt_mm, tag="h_ring")
        nc.vector.memset(h_ring[:], 0.0)

        def whh_g(layer, g):
            return whh_sb[:, (layer * 3 + g) * H:(layer * 3 + g + 1) * H]

        def wih_g(layer, g):
            if layer == 0:
                return wih0_sb[:, g * H:(g + 1) * H]
            base = ((layer - 1) * 3 + g) * H
            return wih_sb[:, base:base + H]

        for w in range(w_end):
            s2 = w % 2
            r4 = w % 4
            slot = w % RING
            for g, grp in enumerate(GRPS):
                if s2 == 0:
                    # ---- refill rz + gxn ring slots for steps w, w+1 ----
                    nc.tensor.matmul(rz_t[g][:, r4:r4 + 2, 0:6, :],
                                     brz_sb[g][:], erz_sb[:],
                                     start=True, stop=False)
                    nc.tensor.matmul(gxn_t[g][:, r4:r4 + 2, 0:3, :],
                                     bihn_sb[g][:], e3_sb[:],
                                     start=True, stop=False)
                    for j, l in enumerate(grp):
                        if l == 0:
                            tc0 = min(w, t_steps - 2)
                            rhs = xT_sb[:, tc0 * PB:(tc0 + 2) * PB]
                        else:
                            ps = (w - D_OFF) % RING  # even, no wrap
                            rhs = h_ring[:, l - 1, ps:ps + 2, :]
                        for gate in range(2):
                            nc.tensor.matmul(rz_t[g][:, r4:r4 + 2, j * 2 + gate, :],
                                             wih_g(l, gate), rhs,
                                             start=False, stop=False,
                                             skip_group_check=True)
                        nc.tensor.matmul(gxn_t[g][:, r4:r4 + 2, j, :],
                                         wih_g(l, 2), rhs,
                                         start=False, stop=True,
                                         skip_group_check=True)

                # ---- per-step recurrent matmuls ----
                ghn = ghn_t[g][:, s2]  # [H, 4, PB] ping-pong slot
                nc.tensor.matmul(ghn[:, 0:3, :], bhhn_sb[g][:], e3b_sb[:],
                                 start=True, stop=False)
                prev_slot = (w - 1) % RING
                for j, l in enumerate(grp):
                    t_l = w - D_OFF * l
                    if t_l == 0:
                        hprev = zeros_sb[:]
                    else:
                        hprev = h_ring[:, l, prev_slot, :]
                    nc.tensor.matmul(ghn[:, j, :], whh_g(l, 2), hprev,
                                     start=False, stop=True,
                                     skip_group_check=True)
                    nc.tensor.matmul(rz_t[g][:, r4, j * 2, :], whh_g(l, 0), hprev,
                                     start=False, stop=True,
                                     skip_group_check=True)
                    nc.tensor.matmul(rz_t[g][:, r4, j * 2 + 1, :], whh_g(l, 1), hprev,
                                     start=False, stop=True,
                                     skip_group_check=True)

                # ---- gates ----
                rz_sb = rzsb_pool.tile([H, 3, 2, PB], dt_mm, tag=f"rzsb{g}",
                                       name=f"rzsb{g}")
                nc.scalar.activation(rz_sb[:], rz_t[g][:, r4, 0:6, :], AF.Sigmoid)
                r_view = rz_sb[:, :, 0, :]
                z_view = rz_sb[:, :, 1, :]
                hn2_sb = ew_pool.tile([H, 3, PB], dt_mm, tag=f"hn2{g}",
                                      name=f"hn2{g}")
                nc.vector.tensor_tensor(hn2_sb[:], ghn[:, 0:3, :], r_view,
                                        op=ALU.mult)
                nin_sb = ew_pool.tile([H, 3, PB], dt_mm, tag=f"nin{g}",
                                      name=f"nin{g}")
                nc.vector.tensor_tensor(nin_sb[:], hn2_sb[:],
                                        gxn_t[g][:, r4, 0:3, :], op=ALU.add)
                n_sb = ew_pool.tile([H, 3, PB], dt_mm, tag=f"n{g}", name=f"n{g}")
                nc.scalar.activation(n_sb[:], nin_sb[:], AF.Tanh)
                # ---- h update: h = n + z*(hprev - n) ----
                hprev3 = h_ring[:, grp[0]:grp[0] + 3, prev_slot, :]
                d_sb = ew_pool.tile([H, 3, PB], dt_mm, tag=f"d{g}", name=f"d{g}")
                nc.gpsimd.tensor_tensor(d_sb[:], hprev3, n_sb[:], op=ALU.subtract)
                e_sb = ew_pool.tile([H, 3, PB], dt_mm, tag=f"e{g}", name=f"e{g}")
                nc.gpsimd.tensor_tensor(e_sb[:], z_view, d_sb[:], op=ALU.mult)
                nc.vector.tensor_tensor(h_ring[:, grp[0]:grp[0] + 3, slot, :],
                                        n_sb[:], e_sb[:], op=ALU.add)
                # zero the slot a layer will read as h(-1) at its t=0
                for l in grp:
                    if l > 0 and w == D_OFF * l - 1:
                        nc.vector.memset(h_ring[:, l, slot, :], 0.0)

        # ---- FC + log_softmax on h(L-1, T-1) ----
        h_last = h_ring[:, L - 1, (w_end - 1) % RING, :]
        # reuse an untouched pad column of group 1's ghn bank for the logits
        logits_ps = ghn_t[1][0:PB, 1, 3, 0:O]
        nc.tensor.matmul(logits_ps, h_last, fcw_sb[:], start=True, stop=False)
        nc.tensor.matmul(logits_ps, ones_sb[:], fcb_sb[:],
                         start=False, stop=True, skip_group_check=True)
        mx_t = scratch.tile([PB, 1], f32, tag="mx")
        nc.vector.reduce_max(mx_t[:], logits_ps, axis=mybir.AxisListType.X)
        xm_t = scratch.tile([PB, O], f32, tag="xm")
        nc.vector.tensor_scalar(xm_t[:], logits_ps, mx_t[:], None,
                                op0=ALU.subtract)
        ex_t = scratch.tile([PB, O], f32, tag="ex")
        sum_t = scratch.tile([PB, 1], f32, tag="sum")
        nc.scalar.activation(ex_t[:], xm_t[:], AF.Exp, accum_out=sum_t[:])
        ls_t = scratch.tile([PB, 1], f32, tag="ls")
        nc.scalar.activation(ls_t[:], sum_t[:], AF.Ln)
        out_t = scratch.tile([PB, O], f32, tag="out")
        nc.vector.tensor_scalar(out_t[:], xm_t[:], ls_t[:], None,
                                op0=ALU.subtract)
        nc.gpsimd.dma_start(y.ap(), out_t[:])

    nc.compile()
    return nc


def _prep_inputs(x, W_ih0, W_ih_rest, W_hh, b_ih, b_hh, fc_w, fc_b, t_steps,
                 np_mm=None):
    """Host-side reshape/transpose into the layouts the kernel expects."""
    import ml_dtypes
    if np_mm is None:
        np_mm = ml_dtypes.bfloat16
    f = np.float32
    b_ih = np.asarray(b_ih, f)
    b_hh = np.asarray(b_hh, f)

    def brz(grp):
        rows = []
        for l in grp:
            for gate in range(2):
                rows.append(b_ih[l, gate * H:(gate + 1) * H]
                            + b_hh[l, gate * H:(gate + 1) * H])
        return np.ascontiguousarray(np.stack(rows).astype(np_mm))

    erz = np.zeros((6, 2 * 3 * 2 * PB), f)
    for k in range(6):
        for s in range(2):
            base = s * (3 * 2 * PB) + k * PB
            erz[k, base:base + PB] = 1.0
    e3 = np.zeros((3, 2 * 3 * PB), f)
    for j in range(3):
        for s in range(2):
            base = s * (3 * PB) + j * PB
            e3[j, base:base + PB] = 1.0
    e3b = np.zeros((3, 3 * PB), f)
    for j in range(3):
        e3b[j, j * PB:(j + 1) * PB] = 1.0

    def bn(arr, grp):
        return np.ascontiguousarray(
            np.stack([arr[l, 2 * H:3 * H] for l in grp]).astype(np_mm))

    shared = {
        "wih0": np.ascontiguousarray(np.asarray(W_ih0, f).T.astype(np_mm)),
        "wih": np.ascontiguousarray(
            np.concatenate([np.asarray(W_ih_rest[l], f).T for l in range(L - 1)],
                           axis=1).astype(np_mm)),
        "whh": np.ascontiguousarray(
            np.concatenate([np.asarray(W_hh[l], f).T for l in range(L)],
                           axis=1).astype(np_mm)),
        "brz_a": brz(GRPS[0]),
        "brz_b": brz(GRPS[1]),
        "erz": np.ascontiguousarray(erz.astype(np_mm)),
        "bihn_a": bn(b_ih, GRPS[0]),
        "bihn_b": bn(b_ih, GRPS[1]),
        "e3": np.ascontiguousarray(e3.astype(np_mm)),
        "bhhn_a": bn(b_hh, GRPS[0]),
        "bhhn_b": bn(b_hh, GRPS[1]),
        "e3b": np.ascontiguousarray(e3b.astype(np_mm)),
        "fcw": np.ascontiguousarray(np.asarray(fc_w, f).T.astype(np_mm)),
        "fcb": np.ascontiguousarray(np.asarray(fc_b, f).reshape(1, O).astype(np_mm)),
    }
    x = np.asarray(x, f)[:, :t_steps, :]
    in_maps = []
    for c in range(NCORES):
        xc = x[c * PB:(c + 1) * PB]                      # [PB, t, I]
        xT_c = np.ascontiguousarray(
            xc.transpose(2, 1, 0).reshape(I_DIM, t_steps * PB).astype(np_mm))
        in_maps.append({"xT": xT_c, **shared})
    return in_maps


def _run(nc, in_maps, trace=False):
    from concourse.bass_utils import run_bass_kernel_spmd
    return run_bass_kernel_spmd(nc, in_maps, core_ids=list(range(NCORES)),
                                trace=trace)


def kernel(x, W_ih0, W_ih_rest, W_hh, b_ih, b_hh, fc_w, fc_b):
    key = ("bf16", T)
    if key not in _CACHE:
        _CACHE[key] = _build(T, "bfloat16")
    nc = _CACHE[key]
    in_maps = _prep_inputs(x, W_ih0, W_ih_rest, W_hh, b_ih, b_hh, fc_w, fc_b, T)
    res = _run(nc, in_maps)
    return np.concatenate([res.results[c]["y"] for c in range(NCORES)], axis=0)


# revision 12
# speedup vs baseline: 3.6851x; 1.0714x over previous
"""Trainium2 Bass kernel for a 6-layer GRU network (B=256, T=512, I=28, H=128, O=10).

Strategy: data-parallel across 8 NeuronCores (batch 256 -> 32 per core),
with a 6-layer WAVEFRONT schedule inside each core: at wavefront step w,
layer l processes timestep t = w - 8*l.  The six layers are split into two
independent groups of three (layers 0-2 / 3-5) whose dependency chains
interleave on the engines, and all gate elementwise work is batched across
each group's three layers into [128, 96]-wide ops (vs [128, 32] per-layer).

Per group-step:
  - PSUM "rz" tile [128, 2steps x 3layers x 2gates x 32] accumulates
    bias (K=6 selector matmul, start=True) + input projection (chunked,
    strided dest, start=False) + recurrent W_hh matmuls (start=False) so
    ONE sigmoid op reads a contiguous [128,192] tile and emits bf16 SBUF.
  - n-gate: gxn PSUM tile (bias + input proj), ghn PSUM tile (bias +
    recurrent mm); hn2 = ghn * r and nin = hn2 + gxn on GpSimd; tanh on
    ScalarE; h-update (d = h-n, e = z*d, h = n+e) on DVE in bf16 SBUF
    (4x fast mode).
  - h state lives in per-layer SBUF rings [128, L, 16, 32] indexed by
    wavefront slot (w % 16), so the batched 3-layer h-update writes one
    strided AP.
Final FC + log_softmax identical to the data-parallel baseline.
"""

import numpy as np

H = 128
I_DIM = 28
L = 6
O = 10
B = 256
T = 512
NCORES = 8
PB = B // NCORES   # 32 batch rows per core
D_OFF = 8          # wavefront offset between consecutive layers
RING = 16          # h-state ring depth (slots of PB cols per layer)
GRPS = ([0, 1, 2], [3, 4, 5])

_CACHE = {}


def _build(t_steps, dt_mm_name="bfloat16"):
    from contextlib import ExitStack

    import concourse.bass as bass  # noqa: F401
    import concourse.tile as tile
    from concourse import bacc, mybir

    f32 = mybir.dt.float32
    bf16 = mybir.dt.bfloat16
    dt_mm = getattr(mybir.dt, dt_mm_name)
    AF = mybir.ActivationFunctionType
    ALU = mybir.AluOpType

    assert t_steps % 2 == 0
    w_end = t_steps + (L - 1) * D_OFF  # wavefront length

    nc = bacc.Bacc("TRN2", target_bir_lowering=False, debug=False)

    xT = nc.dram_tensor("xT", [I_DIM, PB * t_steps], dt_mm, kind="ExternalInput")
    wih0 = nc.dram_tensor("wih0", [I_DIM, 3 * H], dt_mm, kind="ExternalInput")
    wih = nc.dram_tensor("wih", [H, (L - 1) * 3 * H], dt_mm, kind="ExternalInput")
    whh = nc.dram_tensor("whh", [H, L * 3 * H], dt_mm, kind="ExternalInput")
    # rz bias rows per group: [6, H] (row k = layer grp[k//2], gate k%2 (r/z))
    brz_a = nc.dram_tensor("brz_a", [6, H], dt_mm, kind="ExternalInput")
    brz_b = nc.dram_tensor("brz_b", [6, H], dt_mm, kind="ExternalInput")
    erz = nc.dram_tensor("erz", [6, 2 * 3 * 2 * PB], dt_mm, kind="ExternalInput")
    bihn_a = nc.dram_tensor("bihn_a", [3, H], dt_mm, kind="ExternalInput")
    bihn_b = nc.dram_tensor("bihn_b", [3, H], dt_mm, kind="ExternalInput")
    e3 = nc.dram_tensor("e3", [3, 2 * 3 * PB], dt_mm, kind="ExternalInput")
    bhhn_a = nc.dram_tensor("bhhn_a", [3, H], dt_mm, kind="ExternalInput")
    bhhn_b = nc.dram_tensor("bhhn_b", [3, H], dt_mm, kind="ExternalInput")
    fcw = nc.dram_tensor("fcw", [H, O], dt_mm, kind="ExternalInput")
    fcb = nc.dram_tensor("fcb", [1, O], dt_mm, kind="ExternalInput")
    y = nc.dram_tensor("y", [PB, O], f32, kind="ExternalOutput")

    with tile.TileContext(nc) as tc, ExitStack() as ctx:
        consts = ctx.enter_context(tc.tile_pool(name="consts", bufs=1))
        # One persistent PSUM pool per group: rz ring (2 banks) + gxn ring
        # (1 bank) + ghn ping-pong (0.5 bank) = 4 banks; x2 groups = 8 banks.
        # Ring slots are padded so no matmul dest window crosses a bank.
        ps_pool = [
            ctx.enter_context(tc.tile_pool(name=f"ps_pool{g}", bufs=1, space="PSUM"))
            for g in range(2)
        ]
        rz_t = []
        gxn_t = []
        ghn_t = []
        for g in range(2):
            rz = ps_pool[g].tile([H, 4, 8, PB], f32, tag=f"rz{g}", name=f"rz{g}")
            gxn = ps_pool[g].tile([H, 4, 4, PB], f32, tag=f"gxn{g}", name=f"gxn{g}")
            ghn = ps_pool[g].tile([H, 2, 4, PB], f32, tag=f"ghn{g}", name=f"ghn{g}")
            rz_t.append(rz)
            gxn_t.append(gxn)
            ghn_t.append(ghn)
        rzsb_pool = ctx.enter_context(tc.tile_pool(name="rzsb", bufs=3))
        ew_pool = ctx.enter_context(tc.tile_pool(name="ew", bufs=3))
        scratch = ctx.enter_context(tc.tile_pool(name="scratch", bufs=3))

        # ---- load constants ----
        xT_sb = consts.tile([I_DIM, PB * t_steps], dt_mm, tag="xT_sb")
        nc.gpsimd.dma_start(xT_sb[:], xT.ap())
        wih0_sb = consts.tile([I_DIM, 3 * H], dt_mm, tag="wih0_sb")
        nc.gpsimd.dma_start(wih0_sb[:], wih0.ap())
        wih_sb = consts.tile([H, (L - 1) * 3 * H], dt_mm, tag="wih_sb")
        nc.gpsimd.dma_start(wih_sb[:], wih.ap())
        whh_sb = consts.tile([H, L * 3 * H], dt_mm, tag="whh_sb")
        nc.gpsimd.dma_start(whh_sb[:], whh.ap())
        brz_sb = [consts.tile([6, H], dt_mm, tag=f"brz{g}_sb", name=f"brz{g}_sb") for g in range(2)]
        nc.gpsimd.dma_start(brz_sb[0][:], brz_a.ap())
        nc.gpsimd.dma_start(brz_sb[1][:], brz_b.ap())
        erz_sb = consts.tile([6, 2 * 3 * 2 * PB], dt_mm, tag="erz_sb")
        nc.gpsimd.dma_start(erz_sb[:], erz.ap())
        bihn_sb = [consts.tile([3, H], dt_mm, tag=f"bihn{g}_sb", name=f"bihn{g}_sb") for g in range(2)]
        nc.gpsimd.dma_start(bihn_sb[0][:], bihn_a.ap())
        nc.gpsimd.dma_start(bihn_sb[1][:], bihn_b.ap())
        e3_sb = consts.tile([3, 2 * 3 * PB], dt_mm, tag="e3_sb")
        nc.gpsimd.dma_start(e3_sb[:], e3.ap())
        bhhn_sb = [consts.tile([3, H], dt_mm, tag=f"bhhn{g}_sb", name=f"bhhn{g}_sb") for g in range(2)]
        nc.gpsimd.dma_start(bhhn_sb[0][:], bhhn_a.ap())
        nc.gpsimd.dma_start(bhhn_sb[1][:], bhhn_b.ap())
        fcw_sb = consts.tile([H, O], dt_mm, tag="fcw_sb")
        nc.gpsimd.dma_start(fcw_sb[:], fcw.ap())
        fcb_sb = consts.tile([1, O], dt_mm, tag="fcb_sb")
        nc.gpsimd.dma_start(fcb_sb[:], fcb.ap())

        zeros_sb = consts.tile([H, PB], dt_mm, tag="zeros_sb")
        nc.vector.memset(zeros_sb[:], 0.0)
        ones_sb = consts.tile([1, PB], dt_mm, tag="ones_sb")
        nc.vector.memset(ones_sb[:], 1.0)

        # h-state rings: [128, layer, slot, batch]; slot = w % RING
        h_ring = consts.tile([H, L, RING, PB], dt_mm, tag="h_ring")
        nc.vector.memset(h_ring[:], 0.0)

        def whh_g(layer, g):
            return whh_sb[:, (layer * 3 + g) * H:(layer * 3 + g + 1) * H]

        def wih_g(layer, g):
            if layer == 0:
                return wih0_sb[:, g * H:(g + 1) * H]
            base = ((layer - 1) * 3 + g) * H
            return wih_sb[:, base:base + H]

        for w in range(w_end):
            s2 = w % 2
            r4 = w % 4
            slot = w % RING
            for g, grp in enumerate(GRPS):
                if s2 == 0:
                    # ---- refill rz + gxn ring slots for steps w, w+1 ----
                    nc.tensor.matmul(rz_t[g][:, r4:r4 + 2, 0:6, :],
                                     brz_sb[g][:], erz_sb[:],
                                     start=True, stop=False)
                    nc.tensor.matmul(gxn_t[g][:, r4:r4 + 2, 0:3, :],
                                     bihn_sb[g][:], e3_sb[:],
                                     start=True, stop=False)
                    for j, l in enumerate(grp):
                        if l == 0:
                            tc0 = min(w, t_steps - 2)
                            rhs = xT_sb[:, tc0 * PB:(tc0 + 2) * PB]
                        else:
                            ps = (w - D_OFF) % RING  # even, no wrap
                            rhs = h_ring[:, l - 1, ps:ps + 2, :]
                        for gate in range(2):
                            nc.tensor.matmul(rz_t[g][:, r4:r4 + 2, j * 2 + gate, :],
                                             wih_g(l, gate), rhs,
                                             start=False, stop=False,
                                             skip_group_check=True)
                        nc.tensor.matmul(gxn_t[g][:, r4:r4 + 2, j, :],
                                         wih_g(l, 2), rhs,
                                         start=False, stop=True,
                                         skip_group_check=True)

                # ---- per-step recurrent matmuls ----
                # rz mms first: the sigmoid (critical chain) waits only on them
                ghn = ghn_t[g][:, s2]  # [H, 4, PB] ping-pong slot
                if s2 == 0:
                    # pre-bias BOTH ghn slots (e3 has the same layer-select
                    # pattern over 2x96 columns as the gxn bias)
                    nc.tensor.matmul(ghn_t[g][:, :, 0:3, :], bhhn_sb[g][:],
                                     e3_sb[:], start=True, stop=False)
                prev_slot = (w - 1) % RING
                hprevs = []
                for j, l in enumerate(grp):
                    t_l = w - D_OFF * l
                    if t_l == 0:
                        hprev = zeros_sb[:]
                    else:
                        hprev = h_ring[:, l, prev_slot, :]
                    hprevs.append(hprev)
                    nc.tensor.matmul(rz_t[g][:, r4, j * 2, :], whh_g(l, 0), hprev,
                                     start=False, stop=True,
                                     skip_group_check=True)
                    nc.tensor.matmul(rz_t[g][:, r4, j * 2 + 1, :], whh_g(l, 1), hprev,
                                     start=False, stop=True,
                                     skip_group_check=True)
                for j, l in enumerate(grp):
                    nc.tensor.matmul(ghn[:, j, :], whh_g(l, 2), hprevs[j],
                                     start=False, stop=True,
                                     skip_group_check=True)

                # ---- gates ----
                rz_sb = rzsb_pool.tile([H, 3, 2, PB], dt_mm, tag=f"rzsb{g}",
                                       name=f"rzsb{g}")
                nc.scalar.activation(rz_sb[:], rz_t[g][:, r4, 0:6, :], AF.Sigmoid)
                r_view = rz_sb[:, :, 0, :]
                z_view = rz_sb[:, :, 1, :]
                hn2_sb = ew_pool.tile([H, 3, PB], dt_mm, tag=f"hn2{g}",
                                      name=f"hn2{g}")
                nc.vector.tensor_tensor(hn2_sb[:], ghn[:, 0:3, :], r_view,
                                        op=ALU.mult)
                nin_sb = ew_pool.tile([H, 3, PB], dt_mm, tag=f"nin{g}",
                                      name=f"nin{g}")
                nc.vector.tensor_tensor(nin_sb[:], hn2_sb[:],
                                        gxn_t[g][:, r4, 0:3, :], op=ALU.add)
                # off-critical-chain pieces of the h update (Pool, all-SBUF):
                #   u  = z * h_prev,  z' = 1 - z
                hprev3 = h_ring[:, grp[0]:grp[0] + 3, prev_slot, :]
                u_sb = ew_pool.tile([H, 3, PB], dt_mm, tag=f"u{g}", name=f"u{g}")
                nc.gpsimd.tensor_tensor(u_sb[:], z_view, hprev3, op=ALU.mult)
                zc_sb = ew_pool.tile([H, 3, PB], dt_mm, tag=f"zc{g}", name=f"zc{g}")
                nc.gpsimd.tensor_scalar(zc_sb[:], z_view, -1.0, 1.0,
                                        op0=ALU.mult, op1=ALU.add)
                n_sb = ew_pool.tile([H, 3, PB], dt_mm, tag=f"n{g}", name=f"n{g}")
                nc.scalar.activation(n_sb[:], nin_sb[:], AF.Tanh)
                # on-chain tail: h = u + (1-z)*n
                v_sb = ew_pool.tile([H, 3, PB], dt_mm, tag=f"v{g}", name=f"v{g}")
                nc.vector.tensor_tensor(v_sb[:], zc_sb[:], n_sb[:], op=ALU.mult)
                nc.vector.tensor_tensor(h_ring[:, grp[0]:grp[0] + 3, slot, :],
                                        u_sb[:], v_sb[:], op=ALU.add)
                # zero the slot a layer will read as h(-1) at its t=0
                for l in grp:
                    if l > 0 and w == D_OFF * l - 1:
                        nc.vector.memset(h_ring[:, l, slot, :], 0.0)

        # ---- FC + log_softmax on h(L-1, T-1) ----
        h_last = h_ring[:, L - 1, (w_end - 1) % RING, :]
        # reuse an untouched pad column of group 1's ghn bank for the logits
        logits_ps = ghn_t[1][0:PB, 1, 3, 0:O]
        nc.tensor.matmul(logits_ps, h_last, fcw_sb[:], start=True, stop=False)
        nc.tensor.matmul(logits_ps, ones_sb[:], fcb_sb[:],
                         start=False, stop=True, skip_group_check=True)
        mx_t = scratch.tile([PB, 1], f32, tag="mx")
        nc.vector.reduce_max(mx_t[:], logits_ps, axis=mybir.AxisListType.X)
        xm_t = scratch.tile([PB, O], f32, tag="xm")
        nc.vector.tensor_scalar(xm_t[:], logits_ps, mx_t[:], None,
                                op0=ALU.subtract)
        ex_t = scratch.tile([PB, O], f32, tag="ex")
        sum_t = scratch.tile([PB, 1], f32, tag="sum")
        nc.scalar.activation(ex_t[:], xm_t[:], AF.Exp, accum_out=sum_t[:])
        ls_t = scratch.tile([PB, 1], f32, tag="ls")
        nc.scalar.activation(ls_t[:], sum_t[:], AF.Ln)
        out_t = scratch.tile([PB, O], f32, tag="out")
        nc.vector.tensor_scalar(out_t[:], xm_t[:], ls_t[:], None,
                                op0=ALU.subtract)
        nc.gpsimd.dma_start(y.ap(), out_t[:])

    nc.compile()
    return nc


def _prep_inputs(x, W_ih0, W_ih_rest, W_hh, b_ih, b_hh, fc_w, fc_b, t_steps,
                 np_mm=None):
    """Host-side reshape/transpose into the layouts the kernel expects."""
    import ml_dtypes
    if np_mm is None:
        np_mm = ml_dtypes.bfloat16
    f = np.float32
    b_ih = np.asarray(b_ih, f)
    b_hh = np.asarray(b_hh, f)

    def brz(grp):
        rows = []
        for l in grp:
            for gate in range(2):
                rows.append(b_ih[l, gate * H:(gate + 1) * H]
                            + b_hh[l, gate * H:(gate + 1) * H])
        return np.ascontiguousarray(np.stack(rows).astype(np_mm))

    erz = np.zeros((6, 2 * 3 * 2 * PB), f)
    for k in range(6):
        for s in range(2):
            base = s * (3 * 2 * PB) + k * PB
            erz[k, base:base + PB] = 1.0
    e3 = np.zeros((3, 2 * 3 * PB), f)
    for j in range(3):
        for s in range(2):
            base = s * (3 * PB) + j * PB
            e3[j, base:base + PB] = 1.0

    def bn(arr, grp):
        return np.ascontiguousarray(
            np.stack([arr[l, 2 * H:3 * H] for l in grp]).astype(np_mm))

    shared = {
        "wih0": np.ascontiguousarray(np.asarray(W_ih0, f).T.astype(np_mm)),
        "wih": np.ascontiguousarray(
            np.concatenate([np.asarray(W_ih_rest[l], f).T for l in range(L - 1)],
                           axis=1).astype(np_mm)),
        "whh": np.ascontiguousarray(
            np.concatenate([np.asarray(W_hh[l], f).T for l in range(L)],
                           axis=1).astype(np_mm)),
        "brz_a": brz(GRPS[0]),
        "brz_b": brz(GRPS[1]),
        "erz": np.ascontiguousarray(erz.astype(np_mm)),
        "bihn_a": bn(b_ih, GRPS[0]),
        "bihn_b": bn(b_ih, GRPS[1]),
        "e3": np.ascontiguousarray(e3.astype(np_mm)),
        "bhhn_a": bn(b_hh, GRPS[0]),
        "bhhn_b": bn(b_hh, GRPS[1]),
        "fcw": np.ascontiguousarray(np.asarray(fc_w, f).T.astype(np_mm)),
        "fcb": np.ascontiguousarray(np.asarray(fc_b, f).reshape(1, O).astype(np_mm)),
    }
    x = np.asarray(x, f)[:, :t_steps, :]
    in_maps = []
    for c in range(NCORES):
        xc = x[c * PB:(c + 1) * PB]                      # [PB, t, I]
        xT_c = np.ascontiguousarray(
            xc.transpose(2, 1, 0).reshape(I_DIM, t_steps * PB).astype(np_mm))
        in_maps.append({"xT": xT_c, **shared})
    return in_maps


def _run(nc, in_maps, trace=False):
    from concourse.bass_utils import run_bass_kernel_spmd
    return run_bass_kernel_spmd(nc, in_maps, core_ids=list(range(NCORES)),
                                trace=trace)


def kernel(x, W_ih0, W_ih_rest, W_hh, b_ih, b_hh, fc_w, fc_b):
    key = ("bf16", T)
    if key not in _CACHE:
        _CACHE[key] = _build(T, "bfloat16")
    nc = _CACHE[key]
    in_maps = _prep_inputs(x, W_ih0, W_ih_rest, W_hh, b_ih, b_hh, fc_w, fc_b, T)
    res = _run(nc, in_maps)
    return np.concatenate([res.results[c]["y"] for c in range(NCORES)], axis=0)


# revision 14
# speedup vs baseline: 3.7635x; 1.0213x over previous
"""Trainium2 Bass kernel for a 6-layer GRU network (B=256, T=512, I=28, H=128, O=10).

Strategy: data-parallel across 8 NeuronCores (batch 256 -> 32 per core),
with a 6-layer WAVEFRONT schedule inside each core: at wavefront step w,
layer l processes timestep t = w - 8*l.  The six layers are split into two
independent groups of three (layers 0-2 / 3-5) whose dependency chains
interleave on the engines, and all gate elementwise work is batched across
each group's three layers into [128, 96]-wide ops (vs [128, 32] per-layer).

Per group-step:
  - PSUM "rz" tile [128, 2steps x 3layers x 2gates x 32] accumulates
    bias (K=6 selector matmul, start=True) + input projection (chunked,
    strided dest, start=False) + recurrent W_hh matmuls (start=False) so
    ONE sigmoid op reads a contiguous [128,192] tile and emits bf16 SBUF.
  - n-gate: gxn PSUM tile (bias + input proj), ghn PSUM tile (bias +
    recurrent mm); hn2 = ghn * r and nin = hn2 + gxn on GpSimd; tanh on
    ScalarE; h-update (d = h-n, e = z*d, h = n+e) on DVE in bf16 SBUF
    (4x fast mode).
  - h state lives in per-layer SBUF rings [128, L, 16, 32] indexed by
    wavefront slot (w % 16), so the batched 3-layer h-update writes one
    strided AP.
Final FC + log_softmax identical to the data-parallel baseline.
"""

import numpy as np

H = 128
I_DIM = 28
L = 6
O = 10
B = 256
T = 512
NCORES = 8
PB = B // NCORES   # 32 batch rows per core
D_OFF = 8          # wavefront offset between consecutive layers
RING = 16          # h-state ring depth (slots of PB cols per layer)
GRPS = ([0, 1, 2], [3, 4, 5])

_CACHE = {}


def _build(t_steps, dt_mm_name="bfloat16"):
    from contextlib import ExitStack

    import concourse.bass as bass  # noqa: F401
    import concourse.tile as tile
    from concourse import bacc, mybir

    f32 = mybir.dt.float32
    bf16 = mybir.dt.bfloat16
    dt_mm = getattr(mybir.dt, dt_mm_name)
    AF = mybir.ActivationFunctionType
    ALU = mybir.AluOpType

    assert t_steps % 2 == 0
    w_end = t_steps + (L - 1) * D_OFF  # wavefront length

    nc = bacc.Bacc("TRN2", target_bir_lowering=False, debug=False)

    xT = nc.dram_tensor("xT", [I_DIM, PB * t_steps], dt_mm, kind="ExternalInput")
    wih0 = nc.dram_tensor("wih0", [I_DIM, 3 * H], dt_mm, kind="ExternalInput")
    wih = nc.dram_tensor("wih", [H, (L - 1) * 3 * H], dt_mm, kind="ExternalInput")
    whh = nc.dram_tensor("whh", [H, L * 3 * H], dt_mm, kind="ExternalInput")
    # rz bias rows per group: [6, H] (row k = layer grp[k//2], gate k%2 (r/z))
    brz_a = nc.dram_tensor("brz_a", [6, H], dt_mm, kind="ExternalInput")
    brz_b = nc.dram_tensor("brz_b", [6, H], dt_mm, kind="ExternalInput")
    erz = nc.dram_tensor("erz", [6, 2 * 3 * 2 * PB], dt_mm, kind="ExternalInput")
    bihn_a = nc.dram_tensor("bihn_a", [3, H], dt_mm, kind="ExternalInput")
    bihn_b = nc.dram_tensor("bihn_b", [3, H], dt_mm, kind="ExternalInput")
    e3 = nc.dram_tensor("e3", [3, 2 * 3 * PB], dt_mm, kind="ExternalInput")
    bhhn_a = nc.dram_tensor("bhhn_a", [3, H], dt_mm, kind="ExternalInput")
    bhhn_b = nc.dram_tensor("bhhn_b", [3, H], dt_mm, kind="ExternalInput")
    fcw = nc.dram_tensor("fcw", [H, O], dt_mm, kind="ExternalInput")
    fcb = nc.dram_tensor("fcb", [1, O], dt_mm, kind="ExternalInput")
    y = nc.dram_tensor("y", [PB, O], f32, kind="ExternalOutput")

    with tile.TileContext(nc) as tc, ExitStack() as ctx:
        consts = ctx.enter_context(tc.tile_pool(name="consts", bufs=1))
        # One persistent PSUM pool per group: rz ring (2 banks) + gxn ring
        # (1 bank) + ghn ping-pong (0.5 bank) = 4 banks; x2 groups = 8 banks.
        # Ring slots are padded so no matmul dest window crosses a bank.
        ps_pool = [
            ctx.enter_context(tc.tile_pool(name=f"ps_pool{g}", bufs=1, space="PSUM"))
            for g in range(2)
        ]
        rz_t = []
        gxn_t = []
        ghn_t = []
        for g in range(2):
            rz = ps_pool[g].tile([H, 4, 8, PB], f32, tag=f"rz{g}", name=f"rz{g}")
            gxn = ps_pool[g].tile([H, 4, 4, PB], f32, tag=f"gxn{g}", name=f"gxn{g}")
            ghn = ps_pool[g].tile([H, 2, 4, PB], f32, tag=f"ghn{g}", name=f"ghn{g}")
            rz_t.append(rz)
            gxn_t.append(gxn)
            ghn_t.append(ghn)
        rzsb_pool = ctx.enter_context(tc.tile_pool(name="rzsb", bufs=3))
        ew_pool = ctx.enter_context(tc.tile_pool(name="ew", bufs=3))
        scratch = ctx.enter_context(tc.tile_pool(name="scratch", bufs=3))

        # ---- load constants ----
        xT_sb = consts.tile([I_DIM, PB * t_steps], dt_mm, tag="xT_sb")
        nc.gpsimd.dma_start(xT_sb[:], xT.ap())
        wih0_sb = consts.tile([I_DIM, 3 * H], dt_mm, tag="wih0_sb")
        nc.gpsimd.dma_start(wih0_sb[:], wih0.ap())
        wih_sb = consts.tile([H, (L - 1) * 3 * H], dt_mm, tag="wih_sb")
        nc.gpsimd.dma_start(wih_sb[:], wih.ap())
        whh_sb = consts.tile([H, L * 3 * H], dt_mm, tag="whh_sb")
        nc.gpsimd.dma_start(whh_sb[:], whh.ap())
        brz_sb = [consts.tile([6, H], dt_mm, tag=f"brz{g}_sb", name=f"brz{g}_sb") for g in range(2)]
        nc.gpsimd.dma_start(brz_sb[0][:], brz_a.ap())
        nc.gpsimd.dma_start(brz_sb[1][:], brz_b.ap())
        erz_sb = consts.tile([6, 2 * 3 * 2 * PB], dt_mm, tag="erz_sb")
        nc.gpsimd.dma_start(erz_sb[:], erz.ap())
        bihn_sb = [consts.tile([3, H], dt_mm, tag=f"bihn{g}_sb", name=f"bihn{g}_sb") for g in range(2)]
        nc.gpsimd.dma_start(bihn_sb[0][:], bihn_a.ap())
        nc.gpsimd.dma_start(bihn_sb[1][:], bihn_b.ap())
        e3_sb = consts.tile([3, 2 * 3 * PB], dt_mm, tag="e3_sb")
        nc.gpsimd.dma_start(e3_sb[:], e3.ap())
        bhhn_sb = [consts.tile([3, H], dt_mm, tag=f"bhhn{g}_sb", name=f"bhhn{g}_sb") for g in range(2)]
        nc.gpsimd.dma_start(bhhn_sb[0][:], bhhn_a.ap())
        nc.gpsimd.dma_start(bhhn_sb[1][:], bhhn_b.ap())
        fcw_sb = consts.tile([H, O], dt_mm, tag="fcw_sb")
        nc.gpsimd.dma_start(fcw_sb[:], fcw.ap())
        fcb_sb = consts.tile([1, O], dt_mm, tag="fcb_sb")
        nc.gpsimd.dma_start(fcb_sb[:], fcb.ap())

        zeros_sb = consts.tile([H, PB], dt_mm, tag="zeros_sb")
        nc.vector.memset(zeros_sb[:], 0.0)
        ones_sb = consts.tile([1, PB], dt_mm, tag="ones_sb")
        nc.vector.memset(ones_sb[:], 1.0)

        # h-state rings: [128, layer, slot, batch]; slot = w % RING
        h_ring = consts.tile([H, L, RING, PB], dt_mm, tag="h_ring")
        nc.vector.memset(h_ring[:], 0.0)

        def whh_g(layer, g):
            return whh_sb[:, (layer * 3 + g) * H:(layer * 3 + g + 1) * H]

        def wih_g(layer, g):
            if layer == 0:
                return wih0_sb[:, g * H:(g + 1) * H]
            base = ((layer - 1) * 3 + g) * H
            return wih_sb[:, base:base + H]

        for w in range(w_end):
            s2 = w % 2
            r4 = w % 4
            slot = w % RING
            for g, grp in enumerate(GRPS):
                # ---- refill rz + gxn ring slots 2 steps AHEAD (steps wf,
                # wf+1) so refill WAR deps resolve off the critical chain ----
                refills = []
                if s2 == 0:
                    refills.append(w + 2)
                    if w == 0:
                        refills.insert(0, 0)  # bootstrap: slots for steps 0,1
                for wf in refills:
                    rf = wf % 4
                    nc.tensor.matmul(rz_t[g][:, rf:rf + 2, 0:6, :],
                                     brz_sb[g][:], erz_sb[:],
                                     start=True, stop=False)
                    nc.tensor.matmul(gxn_t[g][:, rf:rf + 2, 0:3, :],
                                     bihn_sb[g][:], e3_sb[:],
                                     start=True, stop=False)
                    for j, l in enumerate(grp):
                        if l == 0:
                            tc0 = min(wf, t_steps - 2)
                            rhs = xT_sb[:, tc0 * PB:(tc0 + 2) * PB]
                        else:
                            ps = (wf - D_OFF) % RING  # even, no wrap
                            rhs = h_ring[:, l - 1, ps:ps + 2, :]
                        for gate in range(2):
                            nc.tensor.matmul(rz_t[g][:, rf:rf + 2, j * 2 + gate, :],
                                             wih_g(l, gate), rhs,
                                             start=False, stop=False,
                                             skip_group_check=True)
                        nc.tensor.matmul(gxn_t[g][:, rf:rf + 2, j, :],
                                         wih_g(l, 2), rhs,
                                         start=False, stop=True,
                                         skip_group_check=True)

                # ---- per-step recurrent matmuls ----
                # rz mms first: the sigmoid (critical chain) waits only on them
                ghn = ghn_t[g][:, s2]  # [H, 4, PB] ping-pong slot
                # per-slot bias (slot last read by hn2 two steps ago -> no
                # mid-chain stall); e3[:, 0:96] is the layer-select pattern
                nc.tensor.matmul(ghn[:, 0:3, :], bhhn_sb[g][:],
                                 e3_sb[:, 0:3 * PB], start=True, stop=False)
                prev_slot = (w - 1) % RING
                hprevs = []
                for j, l in enumerate(grp):
                    t_l = w - D_OFF * l
                    if t_l == 0:
                        hprev = zeros_sb[:]
                    else:
                        hprev = h_ring[:, l, prev_slot, :]
                    hprevs.append(hprev)
                    nc.tensor.matmul(rz_t[g][:, r4, j * 2, :], whh_g(l, 0), hprev,
                                     start=False, stop=True,
                                     skip_group_check=True)
                    nc.tensor.matmul(rz_t[g][:, r4, j * 2 + 1, :], whh_g(l, 1), hprev,
                                     start=False, stop=True,
                                     skip_group_check=True)
                for j, l in enumerate(grp):
                    nc.tensor.matmul(ghn[:, j, :], whh_g(l, 2), hprevs[j],
                                     start=False, stop=True,
                                     skip_group_check=True)

                # ---- gates ----
                rz_sb = rzsb_pool.tile([H, 3, 2, PB], dt_mm, tag=f"rzsb{g}",
                                       name=f"rzsb{g}")
                nc.scalar.activation(rz_sb[:], rz_t[g][:, r4, 0:6, :], AF.Sigmoid)
                r_view = rz_sb[:, :, 0, :]
                z_view = rz_sb[:, :, 1, :]
                hn2_sb = ew_pool.tile([H, 3, PB], dt_mm, tag=f"hn2{g}",
                                      name=f"hn2{g}")
                nc.vector.tensor_tensor(hn2_sb[:], ghn[:, 0:3, :], r_view,
                                        op=ALU.mult)
                nin_sb = ew_pool.tile([H, 3, PB], dt_mm, tag=f"nin{g}",
                                      name=f"nin{g}")
                nc.vector.tensor_tensor(nin_sb[:], hn2_sb[:],
                                        gxn_t[g][:, r4, 0:3, :], op=ALU.add)
                # off-critical-chain pieces of the h update (Pool, all-SBUF):
                #   u  = z * h_prev,  z' = 1 - z
                hprev3 = h_ring[:, grp[0]:grp[0] + 3, prev_slot, :]
                u_sb = ew_pool.tile([H, 3, PB], dt_mm, tag=f"u{g}", name=f"u{g}")
                nc.gpsimd.tensor_tensor(u_sb[:], z_view, hprev3, op=ALU.mult)
                zc_sb = ew_pool.tile([H, 3, PB], dt_mm, tag=f"zc{g}", name=f"zc{g}")
                nc.gpsimd.tensor_scalar(zc_sb[:], z_view, -1.0, 1.0,
                                        op0=ALU.mult, op1=ALU.add)
                n_sb = ew_pool.tile([H, 3, PB], dt_mm, tag=f"n{g}", name=f"n{g}")
                nc.scalar.activation(n_sb[:], nin_sb[:], AF.Tanh)
                # on-chain tail: h = u + (1-z)*n
                v_sb = ew_pool.tile([H, 3, PB], dt_mm, tag=f"v{g}", name=f"v{g}")
                nc.vector.tensor_tensor(v_sb[:], zc_sb[:], n_sb[:], op=ALU.mult)
                nc.vector.tensor_tensor(h_ring[:, grp[0]:grp[0] + 3, slot, :],
                                        u_sb[:], v_sb[:], op=ALU.add)
                # zero the slot a layer will read as h(-1) at its t=0
                for l in grp:
                    if l > 0 and w == D_OFF * l - 1:
                        nc.vector.memset(h_ring[:, l, slot, :], 0.0)

        # ---- FC + log_softmax on h(L-1, T-1) ----
        h_last = h_ring[:, L - 1, (w_end - 1) % RING, :]
        # reuse an untouched pad column of group 1's ghn bank for the logits
        logits_ps = ghn_t[1][0:PB, 1, 3, 0:O]
        nc.tensor.matmul(logits_ps, h_last, fcw_sb[:], start=True, stop=False)
        nc.tensor.matmul(logits_ps, ones_sb[:], fcb_sb[:],
                         start=False, stop=True, skip_group_check=True)
        mx_t = scratch.tile([PB, 1], f32, tag="mx")
        nc.vector.reduce_max(mx_t[:], logits_ps, axis=mybir.AxisListType.X)
        xm_t = scratch.tile([PB, O], f32, tag="xm")
        nc.vector.tensor_scalar(xm_t[:], logits_ps, mx_t[:], None,
                                op0=ALU.subtract)
        ex_t = scratch.tile([PB, O], f32, tag="ex")
        sum_t = scratch.tile([PB, 1], f32, tag="sum")
        nc.scalar.activation(ex_t[:], xm_t[:], AF.Exp, accum_out=sum_t[:])
        ls_t = scratch.tile([PB, 1], f32, tag="ls")
        nc.scalar.activation(ls_t[:], sum_t[:], AF.Ln)
        out_t = scratch.tile([PB, O], f32, tag="out")
        nc.vector.tensor_scalar(out_t[:], xm_t[:], ls_t[:], None,
                                op0=ALU.subtract)
        nc.gpsimd.dma_start(y.ap(), out_t[:])

    nc.compile()
    return nc


def _prep_inputs(x, W_ih0, W_ih_rest, W_hh, b_ih, b_hh, fc_w, fc_b, t_steps,
                 np_mm=None):
    """Host-side reshape/transpose into the layouts the kernel expects."""
    import ml_dtypes
    if np_mm is None:
        np_mm = ml_dtypes.bfloat16
    f = np.float32
    b_ih = np.asarray(b_ih, f)
    b_hh = np.asarray(b_hh, f)

    def brz(grp):
        rows = []
        for l in grp:
            for gate in range(2):
                rows.append(b_ih[l, gate * H:(gate + 1) * H]
                            + b_hh[l, gate * H:(gate + 1) * H])
        return np.ascontiguousarray(np.stack(rows).astype(np_mm))

    erz = np.zeros((6, 2 * 3 * 2 * PB), f)
    for k in range(6):
        for s in range(2):
            base = s * (3 * 2 * PB) + k * PB
            erz[k, base:base + PB] = 1.0
    e3 = np.zeros((3, 2 * 3 * PB), f)
    for j in range(3):
        for s in range(2):
            base = s * (3 * PB) + j * PB
            e3[j, base:base + PB] = 1.0

    def bn(arr, grp):
        return np.ascontiguousarray(
            np.stack([arr[l, 2 * H:3 * H] for l in grp]).astype(np_mm))

    shared = {
        "wih0": np.ascontiguousarray(np.asarray(W_ih0, f).T.astype(np_mm)),
        "wih": np.ascontiguousarray(
            np.concatenate([np.asarray(W_ih_rest[l], f).T for l in range(L - 1)],
                           axis=1).astype(np_mm)),
        "whh": np.ascontiguousarray(
            np.concatenate([np.asarray(W_hh[l], f).T for l in range(L)],
                           axis=1).astype(np_mm)),
        "brz_a": brz(GRPS[0]),
        "brz_b": brz(GRPS[1]),
        "erz": np.ascontiguousarray(erz.astype(np_mm)),
        "bihn_a": bn(b_ih, GRPS[0]),
        "bihn_b": bn(b_ih, GRPS[1]),
        "e3": np.ascontiguousarray(e3.astype(np_mm)),
        "bhhn_a": bn(b_hh, GRPS[0]),
        "bhhn_b": bn(b_hh, GRPS[1]),
        "fcw": np.ascontiguousarray(np.asarray(fc_w, f).T.astype(np_mm)),
        "fcb": np.ascontiguousarray(np.asarray(fc_b, f).reshape(1, O).astype(np_mm)),
    }
    x = np.asarray(x, f)[:, :t_steps, :]
    in_maps = []
    for c in range(NCORES):
        xc = x[c * PB:(c + 1) * PB]                      # [PB, t, I]
        xT_c = np.ascontiguousarray(
            xc.transpose(2, 1, 0).reshape(I_DIM, t_steps * PB).astype(np_mm))
        in_maps.append({"xT": xT_c, **shared})
    return in_maps


def _run(nc, in_maps, trace=False):
    from concourse.bass_utils import run_bass_kernel_spmd
    return run_bass_kernel_spmd(nc, in_maps, core_ids=list(range(NCORES)),
                                trace=trace)


def kernel(x, W_ih0, W_ih_rest, W_hh, b_ih, b_hh, fc_w, fc_b):
    key = ("bf16", T)
    if key not in _CACHE:
        _CACHE[key] = _build(T, "bfloat16")
    nc = _CACHE[key]
    in_maps = _prep_inputs(x, W_ih0, W_ih_rest, W_hh, b_ih, b_hh, fc_w, fc_b, T)
    res = _run(nc, in_maps)
    return np.concatenate([res.results[c]["y"] for c in range(NCORES)], axis=0)
